# revision 2
# baseline (speedup 1.0000x reference)
"""Causal self-attention (B=2, N=2048, E=1024, H=16, D=64) on 8 TRN2 cores.

Sharding: core c -> batch b = c//4, head group g = c%4 (4 heads = 256
features per core).  Each core computes its heads' q/k/v projections,
causal attention, and a partial out-projection; the host sums the 4
partials per batch.

Per-core dataflow (feature-major "transposed" layouts throughout):
  xT [E, N] (f32r)  x  wqkvT slices -> q,k as [feat, tok], v as [tok, feat]
  scoresT [ktok, qtok] = k_h^T-chunks x q_h   (PE row-tiled, 2 heads/pass)
  probsT = exp(scoresT/8) in bf16 (ACT), causal triangle masked (DVE)
  attnT [feat, qtok] += v-chunk^T x probsT    (PE col-tiled, 2 heads/pass)
  denom[q] += ones^T x probsT                 (PE col-tiled M=1, 4 heads)
  attnT normalized by PE-broadcast reciprocal; out = attnT^T x woT chunks.
Causal structure skips all fully-masked k-blocks (half the attention
flops); diagonal blocks are computed on their valid q-range only.
"""

import os
import sys
import types

import numpy as np

B, N, E, H, D = 2, 2048, 1024, 16, 64
NCORES = 8


# ---------------------------------------------------------------------------
# Environment patches (this container's walrus accepts only one sync wait per
# instruction; the image's antenv lacks the NTFF profile hook shim).
# ---------------------------------------------------------------------------

def _patch_tile_drain():
    import concourse.mybir as mybir
    import concourse.tile as tile_mod
    from concourse.vector_clock import ScopedClock

    if getattr(tile_mod.TileContext, "_drain_patched", False):
        return

    def _drain_and_barrier(self, tick_clock, wait_clock):
        nc = self.nc
        probe = nc.sync.nop()
        wait_clock.add_sem_waits(probe.ins, ScopedClock({None: tick_clock.global_clock}))
        si = probe.ins.sync_info
        waits = list(si.on_wait) if si and si.on_wait else []
        if len(waits) > 1:
            si.on_wait = waits[:1]
            for w in waits[1:]:
                extra = nc.sync.nop()
                extra.ins.sync_info = mybir.SyncInfo(on_wait=[w], on_update=[])
        nc.sync.drain()
        nc.all_engine_barrier()
        assert self.sems is not None
        popped = nc._tile_sem_poison_stack.pop()
        assert popped is self._sem_poison
        nc.clear_and_free_semaphores(list(self.sems.allocated().values()))
        nc.all_engine_barrier()

    tile_mod.TileContext._drain_and_barrier = _drain_and_barrier
    tile_mod.TileContext._drain_patched = True


def _split_sync_waits(nc, max_waits=1):
    import concourse.mybir as mybir

    cnt = 0
    for f in nc.m.functions:
        for blk in f.blocks:
            insts = blk.instructions
            new = []
            for inst in insts:
                si = inst.sync_info
                waits = list(si.on_wait) if si and si.on_wait else []
                if len(waits) > max_waits:
                    keep = waits[-max_waits:]
                    excess = waits[:-max_waits]
                    for j in range(0, len(excess), max_waits):
                        n = mybir.InstNoOp(name=f"I-ws{cnt}", ins=[], outs=[])
                        cnt += 1
                        n.engine = inst.engine
                        n.sync_info = mybir.SyncInfo(
                            on_wait=excess[j:j + max_waits], on_update=[])
                        new.append(n)
                    si.on_wait = keep
                new.append(inst)
            insts[:] = new
    return cnt


def _install_ntff_shim():
    try:
        import antenv
        if "antenv.axon_hooks" in sys.modules:
            return
        mod = types.ModuleType("antenv.axon_hooks")
        mod._hook = None
        mod.set_axon_ntff_profile_hook = lambda h: setattr(mod, "_hook", h)
        mod.get_axon_ntff_profile_hook = lambda: mod._hook
        sys.modules["antenv.axon_hooks"] = mod
        antenv.axon_hooks = mod
        from trn_agent_boot.trn_boot import _ntff_profile_via_ctypes
        mod._hook = _ntff_profile_via_ctypes("/opt/axon/libaxon_pjrt.so")
    except Exception:
        pass


# ---------------------------------------------------------------------------
# Device program (identical on all 8 cores)
# ---------------------------------------------------------------------------

def _build_nc():
    import concourse.bass as bass
    import concourse.mybir as mybir
    import concourse.tile as tile

    _patch_tile_drain()

    f32 = mybir.dt.float32
    f32r = mybir.dt.float32r
    bf16 = mybir.dt.bfloat16
    AF = mybir.ActivationFunctionType

    nc = bass.Bass("TRN2", target_bir_lowering=False, debug=False)

    xT = nc.dram_tensor("xT", [E, N], f32r, kind="ExternalInput")
    wqkvT = nc.dram_tensor("wqkvT", [E, 768], f32r, kind="ExternalInput")
    woT = nc.dram_tensor("woT", [256, E], f32r, kind="ExternalInput")
    bqkv = nc.dram_tensor("bqkv", [768, 1], f32, kind="ExternalInput")
    tri = nc.dram_tensor("tri", [128, 128], bf16, kind="ExternalInput")
    outp = nc.dram_tensor("outp", [N, E], f32, kind="ExternalOutput")

    NB = N // 512          # 4 token blocks of 512
    NT = N // 128          # 16 token tiles of 128
    NE = E // 128          # 8 contraction chunks

    with nc.allow_low_precision(reason="f32r/bf16 attention pipeline"), \
            tile.TileContext(nc) as tc:
        with tc.tile_pool(name="const", bufs=1) as constp, \
                tc.tile_pool(name="qk", bufs=1) as qkp, \
                tc.tile_pool(name="probs", bufs=4) as pbp, \
                tc.tile_pool(name="misc", bufs=2) as miscp, \
                tc.tile_pool(name="stage", bufs=3) as stp:

            xT_sb = constp.tile([128, NE, N], f32r, tag="xT")
            wq_sb = constp.tile([128, NE, 768], f32r, tag="wq")
            wo_sb = constp.tile([128, 2, E], f32r, tag="wo")
            bias_sb = constp.tile([128, 6, 1], f32, tag="bias")
            tri_sb = constp.tile([128, 128], bf16, tag="tri")
            ones_sb = constp.tile([128, 64], bf16, tag="ones")
            q_sb = qkp.tile([128, 2, N], f32r, tag="q")
            k_sb = qkp.tile([128, 2, N], f32r, tag="k")
            vt_sb = qkp.tile([128, NT, 256], bf16, tag="vt")
            at_sb = qkp.tile([128, 2, N], f32r, tag="at")

            nc.vector.memset(ones_sb[:], 1.0)
            nc.sync.dma_start(tri_sb[:], tri.ap())
            for i in range(NE):
                nc.sync.dma_start(xT_sb[:, i, :], xT.ap()[i * 128:(i + 1) * 128, :])
                nc.sync.dma_start(wq_sb[:, i, :], wqkvT.ap()[i * 128:(i + 1) * 128, :])
            for i in range(2):
                nc.sync.dma_start(wo_sb[:, i, :], woT.ap()[i * 128:(i + 1) * 128, :])
            for i in range(6):
                nc.sync.dma_start(bias_sb[:, i, :], bqkv.ap()[i * 128:(i + 1) * 128, :])

            # ---- phase 1: qkv projections --------------------------------
            with tc.tile_pool(name="p1", bufs=3, space="PSUM") as p1p:
                for nb in range(NB):
                    ts5 = slice(nb * 512, nb * 512 + 512)
                    for ft in range(4):      # q0 q1 k0 k1 feature tiles
                        ps = p1p.tile([128, 512], f32, tag="p1")
                        for e in range(NE):
                            nc.tensor.matmul(
                                ps[:],
                                wq_sb[:, e, ft * 128:(ft + 1) * 128],
                                xT_sb[:, e, ts5],
                                start=(e == 0), stop=(e == NE - 1))
                        dest = (q_sb if ft < 2 else k_sb)[:, ft % 2, ts5]
                        nc.scalar.activation(dest, ps[:], AF.Identity,
                                             bias=bias_sb[:, ft, :], scale=1.0)
                    for tt in range(4 * nb, 4 * nb + 4):   # v token tiles
                        ps = p1p.tile([128, 256], f32, tag="p1")
                        for e in range(NE):
                            nc.tensor.matmul(
                                ps[:],
                                xT_sb[:, e, tt * 128:(tt + 1) * 128],
                                wq_sb[:, e, 512:768],
                                start=(e == 0), stop=(e == NE - 1))
                        nc.vector.tensor_copy(vt_sb[:, tt, :], ps[:])

            # ---- phase 2: causal attention -------------------------------
            with tc.tile_pool(name="sc", bufs=2, space="PSUM") as scp, \
                    tc.tile_pool(name="pv", bufs=1, space="PSUM") as pvp, \
                    tc.tile_pool(name="den", bufs=1, space="PSUM") as denp:
                for j in range(NB):
                    pv_ps = [pvp.tile([128, 512], f32, tag=f"pv{p}",
                                      name=f"pv{p}_{j}") for p in (0, 1)]
                    den_ps = denp.tile([128, 512], f32, tag="den")
                    nk = 4 * (j + 1)
                    for ik in range(nk):
                        r = ik - 4 * j
                        qoff = 128 * r if r > 0 else 0
                        qs = slice(512 * j + qoff, 512 * (j + 1))
                        first, last = ik == 0, ik == nk - 1
                        for p in (0, 1):
                            sc = scp.tile([128, 2, 512], f32, tag="sc")
                            for hh in (0, 1):
                                dsl = slice(64 * hh, 64 * hh + 64)
                                nc.tensor.matmul(
                                    sc[:, hh, qoff:512],
                                    k_sb[dsl, p, ik * 128:(ik + 1) * 128],
                                    q_sb[dsl, p, qs],
                                    start=True, stop=True)
                            pb = pbp.tile([128, 2, 512], bf16, tag="pb")
                            nc.scalar.activation(pb[:, :, qoff:512],
                                                 sc[:, :, qoff:512],
                                                 AF.Exp, scale=float(D) ** -0.5)
                            if r >= 0:
                                for hh in (0, 1):
                                    nc.vector.tensor_mul(
                                        pb[:, hh, qoff:qoff + 128],
                                        pb[:, hh, qoff:qoff + 128], tri_sb[:])
                            for hh in (0, 1):
                                h = 2 * p + hh
                                nc.tensor.matmul(
                                    pv_ps[p][64 * hh:64 * hh + 64, qoff:512],
                                    vt_sb[:, ik, 64 * h:64 * h + 64],
                                    pb[:, hh, qoff:512],
                                    start=first, stop=last,
                                    tile_position=(0, 64 * hh),
                                    skip_group_check=True)
                                nc.tensor.matmul(
                                    den_ps[32 * h:32 * h + 1, qoff:512],
                                    ones_sb[:, 0:1],
                                    pb[:, hh, qoff:512],
                                    start=first, stop=last,
                                    tile_position=(0, 32 * h),
                                    skip_group_check=True)
                    # normalize + v-bias into attnT
                    rec = miscp.tile([128, 512], bf16, tag="rec")
                    for h in range(4):
                        nc.vector.reciprocal(rec[32 * h:32 * h + 1, :],
                                             den_ps[32 * h:32 * h + 1, :])
                    bc = scp.tile([128, 2, 512], f32, tag="sc")
                    for p in (0, 1):
                        for hh in (0, 1):
                            h = 2 * p + hh
                            nc.tensor.matmul(
                                bc[64 * hh:64 * hh + 64, p, :],
                                ones_sb[32 * h:32 * h + 1, 0:64],
                                rec[32 * h:32 * h + 1, :],
                                start=True, stop=True,
                                tile_position=(32 * h, 64 * hh))
                    js = slice(512 * j, 512 * (j + 1))
                    for p in (0, 1):
                        bcs = miscp.tile([128, 512], f32, tag="bcs")
                        nc.vector.tensor_copy(bcs[:], bc[:, p, :])
                        nc.vector.tensor_mul(at_sb[:, p, js], pv_ps[p][:], bcs[:])
                        nc.vector.tensor_scalar_add(at_sb[:, p, js],
                                                    at_sb[:, p, js],
                                                    bias_sb[:, 4 + p, :])

            # ---- phase 3: out-projection (partial) -----------------------
            with tc.tile_pool(name="p3", bufs=3, space="PSUM") as p3p:
                for tt in range(NT):
                    for nb2 in range(2):
                        ps = p3p.tile([128, 512], f32, tag="p3")
                        for fp in range(2):
                            nc.tensor.matmul(
                                ps[:],
                                at_sb[:, fp, tt * 128:(tt + 1) * 128],
                                wo_sb[:, fp, nb2 * 512:(nb2 + 1) * 512],
                                start=(fp == 0), stop=(fp == 1))
                        st = stp.tile([128, 512], f32, tag="st")
                        nc.vector.tensor_copy(st[:], ps[:])
                        nc.sync.dma_start(
                            outp.ap()[tt * 128:(tt + 1) * 128,
                                      nb2 * 512:(nb2 + 1) * 512],
                            st[:])

    _split_sync_waits(nc)
    return nc


_NC = None


def _get_nc():
    global _NC
    if _NC is None:
        _NC = _build_nc()
    return _NC


# ---------------------------------------------------------------------------
# Host entry point
# ---------------------------------------------------------------------------

def kernel(x, qkv_w, qkv_b, out_w, out_b):
    import ml_dtypes
    from concourse.bass_utils import run_bass_kernel_spmd

    trace_dir = os.environ.get("BASS_KERNEL_TRACE_DIR")
    if trace_dir:
        _install_ntff_shim()

    nc = _get_nc()

    x = np.asarray(x, np.float32)
    qkv_w = np.asarray(qkv_w, np.float32)
    qkv_b = np.asarray(qkv_b, np.float32)
    out_w = np.asarray(out_w, np.float32)
    out_b = np.asarray(out_b, np.float32)

    tri_np = np.triu(np.ones((128, 128), np.float32)).astype(ml_dtypes.bfloat16)
    in_maps = []
    for c in range(NCORES):
        b, g = divmod(c, 4)
        fs = slice(256 * g, 256 * g + 256)
        wqkvT = np.ascontiguousarray(
            np.concatenate([qkv_w[0 * E:1 * E][fs],
                            qkv_w[1 * E:2 * E][fs],
                            qkv_w[2 * E:3 * E][fs]], axis=0).T)
        bq = np.concatenate([qkv_b[0 * E:1 * E][fs],
                             qkv_b[1 * E:2 * E][fs],
                             qkv_b[2 * E:3 * E][fs]])[:, None]
        in_maps.append({
            "xT": np.ascontiguousarray(x[b].T),
            "wqkvT": wqkvT,
            "woT": np.ascontiguousarray(out_w[:, fs].T),
            "bqkv": np.ascontiguousarray(bq),
            "tri": tri_np,
        })

    kwargs = {}
    if trace_dir:
        kwargs = {"trace": True, "tmpdir": trace_dir}
    res = run_bass_kernel_spmd(nc, in_maps, core_ids=list(range(NCORES)), **kwargs)
    if trace_dir and res.exec_time_ns is not None:
        print(f"HW exec time: {res.exec_time_ns} ns")

    out = np.zeros((B, N, E), np.float32)
    for c in range(NCORES):
        out[c // 4] += res.results[c]["outp"]
    out += out_b[None, None, :]
    return out


# revision 3
# speedup vs baseline: 1.2256x; 1.2256x over previous
"""Causal self-attention (B=2, N=2048, E=1024, H=16, D=64) on 8 TRN2 cores.

Sharding: core c -> batch b = c//4, head group g = c%4 (4 heads = 256
features per core).  Each core computes its heads' q/k/v projections,
causal attention, and a partial out-projection; the host sums the 4
partials per batch.

Per-core dataflow (feature-major "transposed" layouts throughout):
  xT [E, N] (f16)  x  wqkvT slices -> q,k as [feat, tok], v as [tok, feat]
  scoresT [ktok, qtok] = k_h^T-chunks x q_h   (PE row-tiled, 2 heads/pass)
  probsT = exp(scoresT/8) in f16 (ACT), causal triangle masked (DVE)
  attnT [feat, qtok] += v-chunk^T x probsT    (PE col-tiled, 2 heads/pass)
  denom[q] += ones^T x probsT                 (PE col-tiled M=1, 4 heads)
  attnT normalized by PE-broadcast reciprocal; out = attnT^T x woT chunks.
Causal structure skips all fully-masked k-blocks (half the attention
flops); diagonal blocks are computed on their valid q-range only.
"""

import os
import sys
import types

import numpy as np

B, N, E, H, D = 2, 2048, 1024, 16, 64
NCORES = 8


# ---------------------------------------------------------------------------
# Environment patches (this container's walrus accepts only one sync wait per
# instruction; the image's antenv lacks the NTFF profile hook shim).
# ---------------------------------------------------------------------------

def _patch_tile_drain():
    import concourse.mybir as mybir
    import concourse.tile as tile_mod
    from concourse.vector_clock import ScopedClock

    if getattr(tile_mod.TileContext, "_drain_patched", False):
        return

    def _drain_and_barrier(self, tick_clock, wait_clock):
        nc = self.nc
        probe = nc.sync.nop()
        wait_clock.add_sem_waits(probe.ins, ScopedClock({None: tick_clock.global_clock}))
        si = probe.ins.sync_info
        waits = list(si.on_wait) if si and si.on_wait else []
        if len(waits) > 1:
            si.on_wait = waits[:1]
            for w in waits[1:]:
                extra = nc.sync.nop()
                extra.ins.sync_info = mybir.SyncInfo(on_wait=[w], on_update=[])
        nc.sync.drain()
        nc.all_engine_barrier()
        assert self.sems is not None
        popped = nc._tile_sem_poison_stack.pop()
        assert popped is self._sem_poison
        nc.clear_and_free_semaphores(list(self.sems.allocated().values()))
        nc.all_engine_barrier()

    tile_mod.TileContext._drain_and_barrier = _drain_and_barrier
    tile_mod.TileContext._drain_patched = True


def _split_sync_waits(nc, max_waits=1):
    import concourse.mybir as mybir

    cnt = 0
    for f in nc.m.functions:
        for blk in f.blocks:
            insts = blk.instructions
            new = []
            for inst in insts:
                si = inst.sync_info
                waits = list(si.on_wait) if si and si.on_wait else []
                if len(waits) > max_waits:
                    keep = waits[-max_waits:]
                    excess = waits[:-max_waits]
                    for j in range(0, len(excess), max_waits):
                        n = mybir.InstNoOp(name=f"I-ws{cnt}", ins=[], outs=[])
                        cnt += 1
                        n.engine = inst.engine
                        n.sync_info = mybir.SyncInfo(
                            on_wait=excess[j:j + max_waits], on_update=[])
                        new.append(n)
                    si.on_wait = keep
                new.append(inst)
            insts[:] = new
    return cnt


def _install_ntff_shim():
    try:
        import antenv
        if "antenv.axon_hooks" in sys.modules:
            return
        mod = types.ModuleType("antenv.axon_hooks")
        mod._hook = None
        mod.set_axon_ntff_profile_hook = lambda h: setattr(mod, "_hook", h)
        mod.get_axon_ntff_profile_hook = lambda: mod._hook
        sys.modules["antenv.axon_hooks"] = mod
        antenv.axon_hooks = mod
        from trn_agent_boot.trn_boot import _ntff_profile_via_ctypes
        mod._hook = _ntff_profile_via_ctypes("/opt/axon/libaxon_pjrt.so")
    except Exception:
        pass


# ---------------------------------------------------------------------------
# Device program (identical on all 8 cores)
# ---------------------------------------------------------------------------

def _build_nc():
    import concourse.bass as bass
    import concourse.mybir as mybir
    import concourse.tile as tile

    _patch_tile_drain()

    f32 = mybir.dt.float32
    f16 = mybir.dt.float16
    
    AF = mybir.ActivationFunctionType

    nc = bass.Bass("TRN2", target_bir_lowering=False, debug=False)

    xT = nc.dram_tensor("xT", [E, N], f16, kind="ExternalInput")
    wqkvT = nc.dram_tensor("wqkvT", [E, 768], f16, kind="ExternalInput")
    woT = nc.dram_tensor("woT", [256, E], f16, kind="ExternalInput")
    bqkv = nc.dram_tensor("bqkv", [768, 1], f32, kind="ExternalInput")
    tri = nc.dram_tensor("tri", [128, 128], f16, kind="ExternalInput")
    outp = nc.dram_tensor("outp", [N, E], f32, kind="ExternalOutput")

    NB = N // 512          # 4 token blocks of 512
    NT = N // 128          # 16 token tiles of 128
    NE = E // 128          # 8 contraction chunks

    with nc.allow_low_precision(reason="fp16 matmul pipeline"), \
            tile.TileContext(nc) as tc:
        with tc.tile_pool(name="const", bufs=1) as constp, \
                tc.tile_pool(name="qk", bufs=1) as qkp, \
                tc.tile_pool(name="probs", bufs=4) as pbp, \
                tc.tile_pool(name="misc", bufs=2) as miscp, \
                tc.tile_pool(name="stage", bufs=3) as stp:

            xT_sb = constp.tile([128, NE, N], f16, tag="xT")
            wq_sb = constp.tile([128, NE, 768], f16, tag="wq")
            wo_sb = constp.tile([128, 2, E], f16, tag="wo")
            bias_sb = constp.tile([128, 6, 1], f32, tag="bias")
            tri_sb = constp.tile([128, 128], f16, tag="tri")
            ones_sb = constp.tile([128, 64], f16, tag="ones")
            q_sb = qkp.tile([128, 2, N], f16, tag="q")
            k_sb = qkp.tile([128, 2, N], f16, tag="k")
            vt_sb = qkp.tile([128, NT, 256], f16, tag="vt")
            at_sb = qkp.tile([128, 2, N], f16, tag="at")

            nc.vector.memset(ones_sb[:], 1.0)
            nc.sync.dma_start(tri_sb[:], tri.ap())
            for i in range(NE):
                nc.sync.dma_start(xT_sb[:, i, :], xT.ap()[i * 128:(i + 1) * 128, :])
                nc.sync.dma_start(wq_sb[:, i, :], wqkvT.ap()[i * 128:(i + 1) * 128, :])
            for i in range(2):
                nc.sync.dma_start(wo_sb[:, i, :], woT.ap()[i * 128:(i + 1) * 128, :])
            for i in range(6):
                nc.sync.dma_start(bias_sb[:, i, :], bqkv.ap()[i * 128:(i + 1) * 128, :])

            # ---- phase 1: qkv projections --------------------------------
            with tc.tile_pool(name="p1", bufs=3, space="PSUM") as p1p:
                for nb in range(NB):
                    ts5 = slice(nb * 512, nb * 512 + 512)
                    for ft in range(4):      # q0 q1 k0 k1 feature tiles
                        ps = p1p.tile([128, 512], f32, tag="p1")
                        for e in range(NE):
                            nc.tensor.matmul(
                                ps[:],
                                wq_sb[:, e, ft * 128:(ft + 1) * 128],
                                xT_sb[:, e, ts5],
                                start=(e == 0), stop=(e == NE - 1))
                        dest = (q_sb if ft < 2 else k_sb)[:, ft % 2, ts5]
                        nc.scalar.activation(dest, ps[:], AF.Identity,
                                             bias=bias_sb[:, ft, :], scale=1.0)
                    for tt in range(4 * nb, 4 * nb + 4):   # v token tiles
                        ps = p1p.tile([128, 256], f32, tag="p1")
                        for e in range(NE):
                            nc.tensor.matmul(
                                ps[:],
                                xT_sb[:, e, tt * 128:(tt + 1) * 128],
                                wq_sb[:, e, 512:768],
                                start=(e == 0), stop=(e == NE - 1))
                        nc.vector.tensor_copy(vt_sb[:, tt, :], ps[:])

            # ---- phase 2: causal attention -------------------------------
            with tc.tile_pool(name="sc", bufs=2, space="PSUM") as scp, \
                    tc.tile_pool(name="pv", bufs=1, space="PSUM") as pvp, \
                    tc.tile_pool(name="den", bufs=1, space="PSUM") as denp:
                for j in range(NB):
                    pv_ps = [pvp.tile([128, 512], f32, tag=f"pv{p}",
                                      name=f"pv{p}_{j}") for p in (0, 1)]
                    den_ps = denp.tile([128, 512], f32, tag="den")
                    nk = 4 * (j + 1)
                    for ik in range(nk):
                        r = ik - 4 * j
                        qoff = 128 * r if r > 0 else 0
                        qs = slice(512 * j + qoff, 512 * (j + 1))
                        first, last = ik == 0, ik == nk - 1
                        for p in (0, 1):
                            sc = scp.tile([128, 2, 512], f32, tag="sc")
                            for hh in (0, 1):
                                dsl = slice(64 * hh, 64 * hh + 64)
                                nc.tensor.matmul(
                                    sc[:, hh, qoff:512],
                                    k_sb[dsl, p, ik * 128:(ik + 1) * 128],
                                    q_sb[dsl, p, qs],
                                    start=True, stop=True)
                            pb = pbp.tile([128, 2, 512], f16, tag="pb")
                            nc.scalar.activation(pb[:, :, qoff:512],
                                                 sc[:, :, qoff:512],
                                                 AF.Exp, scale=float(D) ** -0.5)
                            if r >= 0:
                                for hh in (0, 1):
                                    nc.vector.tensor_mul(
                                        pb[:, hh, qoff:qoff + 128],
                                        pb[:, hh, qoff:qoff + 128], tri_sb[:])
                            for hh in (0, 1):
                                h = 2 * p + hh
                                nc.tensor.matmul(
                                    pv_ps[p][64 * hh:64 * hh + 64, qoff:512],
                                    vt_sb[:, ik, 64 * h:64 * h + 64],
                                    pb[:, hh, qoff:512],
                                    start=first, stop=last,
                                    tile_position=(0, 64 * hh),
                                    skip_group_check=True)
                                nc.tensor.matmul(
                                    den_ps[32 * h:32 * h + 1, qoff:512],
                                    ones_sb[:, 0:1],
                                    pb[:, hh, qoff:512],
                                    start=first, stop=last,
                                    tile_position=(0, 32 * h),
                                    skip_group_check=True)
                    # normalize + v-bias into attnT
                    rec = miscp.tile([128, 512], f16, tag="rec")
                    nc.vector.reciprocal(rec[0:97, :], den_ps[0:97, :])
                    bc = scp.tile([128, 2, 512], f32, tag="sc")
                    for p in (0, 1):
                        for hh in (0, 1):
                            h = 2 * p + hh
                            nc.tensor.matmul(
                                bc[64 * hh:64 * hh + 64, p, :],
                                ones_sb[32 * h:32 * h + 1, 0:64],
                                rec[32 * h:32 * h + 1, :],
                                start=True, stop=True,
                                tile_position=(32 * h, 64 * hh))
                    js = slice(512 * j, 512 * (j + 1))
                    for p in (0, 1):
                        bcs = miscp.tile([128, 512], f32, tag="bcs")
                        nc.vector.tensor_copy(bcs[:], bc[:, p, :])
                        nc.vector.tensor_mul(at_sb[:, p, js], pv_ps[p][:], bcs[:])
                        nc.vector.tensor_scalar_add(at_sb[:, p, js],
                                                    at_sb[:, p, js],
                                                    bias_sb[:, 4 + p, :])

            # ---- phase 3: out-projection (partial) -----------------------
            with tc.tile_pool(name="p3", bufs=3, space="PSUM") as p3p:
                for tt in range(NT):
                    for nb2 in range(2):
                        ps = p3p.tile([128, 512], f32, tag="p3")
                        for fp in range(2):
                            nc.tensor.matmul(
                                ps[:],
                                at_sb[:, fp, tt * 128:(tt + 1) * 128],
                                wo_sb[:, fp, nb2 * 512:(nb2 + 1) * 512],
                                start=(fp == 0), stop=(fp == 1))
                        st = stp.tile([128, 512], f32, tag="st")
                        nc.vector.tensor_copy(st[:], ps[:])
                        nc.sync.dma_start(
                            outp.ap()[tt * 128:(tt + 1) * 128,
                                      nb2 * 512:(nb2 + 1) * 512],
                            st[:])

    _split_sync_waits(nc)
    return nc


_NC = None


def _get_nc():
    global _NC
    if _NC is None:
        _NC = _build_nc()
    return _NC


# ---------------------------------------------------------------------------
# Host entry point
# ---------------------------------------------------------------------------

def kernel(x, qkv_w, qkv_b, out_w, out_b):
    from concourse.bass_utils import run_bass_kernel_spmd

    trace_dir = os.environ.get("BASS_KERNEL_TRACE_DIR")
    if trace_dir:
        _install_ntff_shim()

    nc = _get_nc()

    x = np.asarray(x, np.float32)
    qkv_w = np.asarray(qkv_w, np.float32)
    qkv_b = np.asarray(qkv_b, np.float32)
    out_w = np.asarray(out_w, np.float32)
    out_b = np.asarray(out_b, np.float32)

    tri_np = np.triu(np.ones((128, 128), np.float16))
    in_maps = []
    for c in range(NCORES):
        b, g = divmod(c, 4)
        fs = slice(256 * g, 256 * g + 256)
        wqkvT = np.ascontiguousarray(
            np.concatenate([qkv_w[0 * E:1 * E][fs],
                            qkv_w[1 * E:2 * E][fs],
                            qkv_w[2 * E:3 * E][fs]], axis=0).T)
        bq = np.concatenate([qkv_b[0 * E:1 * E][fs],
                             qkv_b[1 * E:2 * E][fs],
                             qkv_b[2 * E:3 * E][fs]])[:, None]
        in_maps.append({
            "xT": np.ascontiguousarray(x[b].T).astype(np.float16),
            "wqkvT": wqkvT.astype(np.float16),
            "woT": np.ascontiguousarray(out_w[:, fs].T).astype(np.float16),
            "bqkv": np.ascontiguousarray(bq),
            "tri": tri_np,
        })

    kwargs = {}
    if trace_dir:
        kwargs = {"trace": True, "tmpdir": trace_dir}
    res = run_bass_kernel_spmd(nc, in_maps, core_ids=list(range(NCORES)), **kwargs)
    if trace_dir and res.exec_time_ns is not None:
        print(f"HW exec time: {res.exec_time_ns} ns")

    out = np.zeros((B, N, E), np.float32)
    for c in range(NCORES):
        out[c // 4] += res.results[c]["outp"]
    out += out_b[None, None, :]
    return out


# revision 5
# speedup vs baseline: 1.5406x; 1.2570x over previous
"""Causal self-attention (B=2, N=2048, E=1024, H=16, D=64) on 8 TRN2 cores.

Sharding: core c -> batch b = c//4, head group g = c%4 (4 heads = 256
features per core).  Each core computes its heads' q/k/v projections,
causal attention, and a partial out-projection; the host sums the 4
partials per batch.

Per-core dataflow (feature-major "transposed" layouts throughout):
  xT [E, N] (f16)  x  wqkvT slices -> q,k as [feat, tok], v as [tok, feat]
  scoresT [ktok, qtok] = k_h^T-chunks x q_h   (PE row-tiled, 2 heads/pass)
  probsT = exp(scoresT/8) in f16 (ACT), causal triangle masked (DVE)
  attnT [feat, qtok] += v-chunk^T x probsT    (PE col-tiled, 2 heads/pass)
  denom[q] += ones^T x probsT                 (PE col-tiled M=1, 4 heads)
  attnT normalized by PE-broadcast reciprocal; out = attnT^T x woT chunks.
Causal structure skips all fully-masked k-blocks (half the attention
flops); diagonal blocks are computed on their valid q-range only.
"""

import os
import sys
import types

import numpy as np

B, N, E, H, D = 2, 2048, 1024, 16, 64
NCORES = 8


# ---------------------------------------------------------------------------
# Environment patches (this container's walrus accepts only one sync wait per
# instruction; the image's antenv lacks the NTFF profile hook shim).
# ---------------------------------------------------------------------------

def _patch_tile_drain():
    import concourse.mybir as mybir
    import concourse.tile as tile_mod
    from concourse.vector_clock import ScopedClock

    if getattr(tile_mod.TileContext, "_drain_patched", False):
        return

    def _drain_and_barrier(self, tick_clock, wait_clock):
        nc = self.nc
        probe = nc.sync.nop()
        wait_clock.add_sem_waits(probe.ins, ScopedClock({None: tick_clock.global_clock}))
        si = probe.ins.sync_info
        waits = list(si.on_wait) if si and si.on_wait else []
        if len(waits) > 1:
            si.on_wait = waits[:1]
            for w in waits[1:]:
                extra = nc.sync.nop()
                extra.ins.sync_info = mybir.SyncInfo(on_wait=[w], on_update=[])
        nc.sync.drain()
        nc.all_engine_barrier()
        assert self.sems is not None
        popped = nc._tile_sem_poison_stack.pop()
        assert popped is self._sem_poison
        nc.clear_and_free_semaphores(list(self.sems.allocated().values()))
        nc.all_engine_barrier()

    tile_mod.TileContext._drain_and_barrier = _drain_and_barrier
    tile_mod.TileContext._drain_patched = True


def _split_sync_waits(nc, max_waits=1):
    import concourse.mybir as mybir

    cnt = 0
    for f in nc.m.functions:
        for blk in f.blocks:
            insts = blk.instructions
            new = []
            for inst in insts:
                si = inst.sync_info
                waits = list(si.on_wait) if si and si.on_wait else []
                if len(waits) > max_waits:
                    keep = waits[-max_waits:]
                    excess = waits[:-max_waits]
                    for j in range(0, len(excess), max_waits):
                        n = mybir.InstNoOp(name=f"I-ws{cnt}", ins=[], outs=[])
                        cnt += 1
                        n.engine = inst.engine
                        n.sync_info = mybir.SyncInfo(
                            on_wait=excess[j:j + max_waits], on_update=[])
                        new.append(n)
                    si.on_wait = keep
                new.append(inst)
            insts[:] = new
    return cnt


def _install_ntff_shim():
    try:
        import antenv
        if "antenv.axon_hooks" in sys.modules:
            return
        mod = types.ModuleType("antenv.axon_hooks")
        mod._hook = None
        mod.set_axon_ntff_profile_hook = lambda h: setattr(mod, "_hook", h)
        mod.get_axon_ntff_profile_hook = lambda: mod._hook
        sys.modules["antenv.axon_hooks"] = mod
        antenv.axon_hooks = mod
        from trn_agent_boot.trn_boot import _ntff_profile_via_ctypes
        mod._hook = _ntff_profile_via_ctypes("/opt/axon/libaxon_pjrt.so")
    except Exception:
        pass


# ---------------------------------------------------------------------------
# Device program (identical on all 8 cores)
# ---------------------------------------------------------------------------

def _build_nc():
    import concourse.bass as bass
    import concourse.mybir as mybir
    import concourse.tile as tile

    _patch_tile_drain()

    f32 = mybir.dt.float32
    f16 = mybir.dt.float16
    
    AF = mybir.ActivationFunctionType

    nc = bass.Bass("TRN2", target_bir_lowering=False, debug=False)

    xT = nc.dram_tensor("xT", [E, N], f16, kind="ExternalInput")
    wqkvT = nc.dram_tensor("wqkvT", [E, 768], f16, kind="ExternalInput")
    woT = nc.dram_tensor("woT", [256, E], f16, kind="ExternalInput")
    bqkv = nc.dram_tensor("bqkv", [768, 1], f32, kind="ExternalInput")
    tri = nc.dram_tensor("tri", [128, 128], f16, kind="ExternalInput")
    outp = nc.dram_tensor("outp", [N, E], f32, kind="ExternalOutput")

    NB = N // 512          # 4 token blocks of 512
    NT = N // 128          # 16 token tiles of 128
    NE = E // 128          # 8 contraction chunks

    with nc.allow_low_precision(reason="fp16 matmul pipeline"), \
            tile.TileContext(nc) as tc:
        with tc.tile_pool(name="const", bufs=1) as constp, \
                tc.tile_pool(name="qk", bufs=1) as qkp, \
                tc.tile_pool(name="probs", bufs=4) as pbp, \
                tc.tile_pool(name="misc", bufs=2) as miscp, \
                tc.tile_pool(name="stage", bufs=3) as stp:

            xT_sb = constp.tile([128, NE, N], f16, tag="xT")
            wq_sb = constp.tile([128, NE, 768], f16, tag="wq")
            wo_sb = constp.tile([128, 2, E], f16, tag="wo")
            bias_sb = constp.tile([128, 6, 1], f32, tag="bias")
            tri_sb = constp.tile([128, 128], f16, tag="tri")
            ones_sb = constp.tile([128, 64], f16, tag="ones")
            q_sb = qkp.tile([128, 2, N], f16, tag="q")
            k_sb = qkp.tile([128, 2, N], f16, tag="k")
            vt_sb = qkp.tile([128, NT, 256], f16, tag="vt")
            at_sb = qkp.tile([128, 2, N], f16, tag="at")

            nc.vector.memset(ones_sb[:], 1.0)
            nc.sync.dma_start(tri_sb[:], tri.ap())
            for i in range(NE):
                nc.sync.dma_start(xT_sb[:, i, :], xT.ap()[i * 128:(i + 1) * 128, :])
                nc.sync.dma_start(wq_sb[:, i, :], wqkvT.ap()[i * 128:(i + 1) * 128, :])
            for i in range(2):
                nc.sync.dma_start(wo_sb[:, i, :], woT.ap()[i * 128:(i + 1) * 128, :])
            for i in range(6):
                nc.sync.dma_start(bias_sb[:, i, :], bqkv.ap()[i * 128:(i + 1) * 128, :])

            # ---- phase 1: qkv projections --------------------------------
            with tc.tile_pool(name="p1", bufs=3, space="PSUM") as p1p:
                for nb in range(NB):
                    ts5 = slice(nb * 512, nb * 512 + 512)
                    for ft in range(4):      # q0 q1 k0 k1 feature tiles
                        ps = p1p.tile([128, 512], f32, tag="p1")
                        for e in range(NE):
                            nc.tensor.matmul(
                                ps[:],
                                wq_sb[:, e, ft * 128:(ft + 1) * 128],
                                xT_sb[:, e, ts5],
                                start=(e == 0), stop=(e == NE - 1))
                        dest = (q_sb if ft < 2 else k_sb)[:, ft % 2, ts5]
                        nc.scalar.activation(dest, ps[:], AF.Identity,
                                             bias=bias_sb[:, ft, :], scale=1.0)
                    for tt in range(4 * nb, 4 * nb + 4):   # v token tiles
                        ps = p1p.tile([128, 256], f32, tag="p1")
                        for e in range(NE):
                            nc.tensor.matmul(
                                ps[:],
                                xT_sb[:, e, tt * 128:(tt + 1) * 128],
                                wq_sb[:, e, 512:768],
                                start=(e == 0), stop=(e == NE - 1))
                        nc.vector.tensor_copy(vt_sb[:, tt, :], ps[:])

            # ---- phase 2: causal attention -------------------------------
            with tc.tile_pool(name="sc", bufs=2, space="PSUM") as scp, \
                    tc.tile_pool(name="pv", bufs=1, space="PSUM") as pvp, \
                    tc.tile_pool(name="den", bufs=1, space="PSUM") as denp:
                for j in range(NB):
                    pv_ps = [pvp.tile([128, 512], f32, tag=f"pv{p}",
                                      name=f"pv{p}_{j}") for p in (0, 1)]
                    den_ps = denp.tile([128, 512], f32, tag="den")
                    nk = 4 * (j + 1)
                    for ik in range(nk):
                        r = ik - 4 * j
                        qoff = 128 * r if r > 0 else 0
                        qs = slice(512 * j + qoff, 512 * (j + 1))
                        first, last = ik == 0, ik == nk - 1
                        pbs = []
                        for p in (0, 1):
                            sc = scp.tile([128, 2, 512], f32, tag="sc")
                            for hh in (0, 1):
                                dsl = slice(64 * hh, 64 * hh + 64)
                                nc.tensor.matmul(
                                    sc[:, hh, qoff:512],
                                    k_sb[dsl, p, ik * 128:(ik + 1) * 128],
                                    q_sb[dsl, p, qs],
                                    start=True, stop=True)
                            pb = pbp.tile([128, 2, 512], f16, tag="pb",
                                          name=f"pb_{j}_{ik}_{p}")
                            nc.scalar.activation(pb[:, :, qoff:512],
                                                 sc[:, :, qoff:512],
                                                 AF.Exp, scale=float(D) ** -0.5)
                            if r >= 0:
                                for hh in (0, 1):
                                    nc.vector.tensor_mul(
                                        pb[:, hh, qoff:qoff + 128],
                                        pb[:, hh, qoff:qoff + 128], tri_sb[:])
                            for hh in (0, 1):
                                h = 2 * p + hh
                                nc.tensor.matmul(
                                    pv_ps[p][64 * hh:64 * hh + 64, qoff:512],
                                    vt_sb[:, ik, 64 * h:64 * h + 64],
                                    pb[:, hh, qoff:512],
                                    start=first, stop=last,
                                    tile_position=(0, 64 * hh),
                                    skip_group_check=True)
                            pbs.append(pb)
                        for h in range(4):
                            nc.tensor.matmul(
                                den_ps[32 * h:32 * h + 1, qoff:512],
                                ones_sb[:, 0:1],
                                pbs[h // 2][:, h % 2, qoff:512],
                                start=first, stop=last,
                                tile_position=(0, 32 * h),
                                skip_group_check=True)
                    # drain PV psum quickly (keeps PE fed), normalize off the
                    # critical path from SBUF copies
                    araw = []
                    for p in (0, 1):
                        ar = miscp.tile([128, 512], f32, tag=f"araw{p}",
                                        name=f"araw{p}_{j}")
                        nc.vector.tensor_copy(ar[:], pv_ps[p][:])
                        araw.append(ar)
                    rec = miscp.tile([128, 512], f16, tag="rec")
                    nc.vector.reciprocal(rec[0:97, :], den_ps[0:97, :])
                    bc = scp.tile([128, 2, 512], f32, tag="sc")
                    for p in (0, 1):
                        for hh in (0, 1):
                            h = 2 * p + hh
                            nc.tensor.matmul(
                                bc[64 * hh:64 * hh + 64, p, :],
                                ones_sb[32 * h:32 * h + 1, 0:64],
                                rec[32 * h:32 * h + 1, :],
                                start=True, stop=True,
                                tile_position=(32 * h, 64 * hh))
                    js = slice(512 * j, 512 * (j + 1))
                    for p in (0, 1):
                        nc.vector.tensor_mul(at_sb[:, p, js], araw[p][:],
                                             bc[:, p, :])
                        nc.vector.tensor_scalar_add(at_sb[:, p, js],
                                                    at_sb[:, p, js],
                                                    bias_sb[:, 4 + p, :])

            # ---- phase 3: out-projection (partial) -----------------------
            with tc.tile_pool(name="p3", bufs=3, space="PSUM") as p3p:
                for tt in range(NT):
                    for nb2 in range(2):
                        ps = p3p.tile([128, 512], f32, tag="p3")
                        for fp in range(2):
                            nc.tensor.matmul(
                                ps[:],
                                at_sb[:, fp, tt * 128:(tt + 1) * 128],
                                wo_sb[:, fp, nb2 * 512:(nb2 + 1) * 512],
                                start=(fp == 0), stop=(fp == 1))
                        st = stp.tile([128, 512], f32, tag="st")
                        nc.vector.tensor_copy(st[:], ps[:])
                        nc.sync.dma_start(
                            outp.ap()[tt * 128:(tt + 1) * 128,
                                      nb2 * 512:(nb2 + 1) * 512],
                            st[:])

    _split_sync_waits(nc)
    return nc


_NC = None


def _get_nc():
    global _NC
    if _NC is None:
        _NC = _build_nc()
    return _NC


# ---------------------------------------------------------------------------
# Host entry point
# ---------------------------------------------------------------------------

def kernel(x, qkv_w, qkv_b, out_w, out_b):
    from concourse.bass_utils import run_bass_kernel_spmd

    trace_dir = os.environ.get("BASS_KERNEL_TRACE_DIR")
    if trace_dir:
        _install_ntff_shim()

    nc = _get_nc()

    x = np.asarray(x, np.float32)
    qkv_w = np.asarray(qkv_w, np.float32)
    qkv_b = np.asarray(qkv_b, np.float32)
    out_w = np.asarray(out_w, np.float32)
    out_b = np.asarray(out_b, np.float32)

    tri_np = np.triu(np.ones((128, 128), np.float16))
    in_maps = []
    for c in range(NCORES):
        b, g = divmod(c, 4)
        fs = slice(256 * g, 256 * g + 256)
        wqkvT = np.ascontiguousarray(
            np.concatenate([qkv_w[0 * E:1 * E][fs],
                            qkv_w[1 * E:2 * E][fs],
                            qkv_w[2 * E:3 * E][fs]], axis=0).T)
        bq = np.concatenate([qkv_b[0 * E:1 * E][fs],
                             qkv_b[1 * E:2 * E][fs],
                             qkv_b[2 * E:3 * E][fs]])[:, None]
        in_maps.append({
            "xT": np.ascontiguousarray(x[b].T).astype(np.float16),
            "wqkvT": wqkvT.astype(np.float16),
            "woT": np.ascontiguousarray(out_w[:, fs].T).astype(np.float16),
            "bqkv": np.ascontiguousarray(bq),
            "tri": tri_np,
        })

    kwargs = {}
    if trace_dir:
        kwargs = {"trace": True, "tmpdir": trace_dir}
    res = run_bass_kernel_spmd(nc, in_maps, core_ids=list(range(NCORES)), **kwargs)
    if trace_dir and res.exec_time_ns is not None:
        print(f"HW exec time: {res.exec_time_ns} ns")

    out = np.zeros((B, N, E), np.float32)
    for c in range(NCORES):
        out[c // 4] += res.results[c]["outp"]
    out += out_b[None, None, :]
    return out


# revision 10
# speedup vs baseline: 1.6408x; 1.0650x over previous
"""Causal self-attention (B=2, N=2048, E=1024, H=16, D=64) on 8 TRN2 cores.

Sharding: core c -> batch b = c//4, head group g = c%4 (4 heads = 256
features per core).  Each core computes its heads' q/k/v projections,
causal attention, and a partial out-projection; the host sums the 4
partials per batch.

Per-core dataflow (feature-major "transposed" layouts throughout):
  xT [E, N] (f16)  x  wqkvT slices -> q,k as [feat, tok], v as [tok, feat]
  scoresT [ktok, qtok] = k_h^T-chunks x q_h   (PE row-tiled, 2 heads/pass)
  probsT = exp(scoresT/8) in f16 (ACT), causal triangle masked (DVE)
  attnT [feat, qtok] += v-chunk^T x probsT    (PE col-tiled, 2 heads/pass)
  denom[q] += ones^T x probsT                 (PE col-tiled M=1, 4 heads)
  attnT normalized by PE-broadcast reciprocal; out = attnT^T x woT chunks.
Causal structure skips all fully-masked k-blocks (half the attention
flops); diagonal blocks are computed on their valid q-range only.
"""

import os
import sys
import types

import numpy as np

B, N, E, H, D = 2, 2048, 1024, 16, 64
NCORES = 8


# ---------------------------------------------------------------------------
# Environment patches (this container's walrus accepts only one sync wait per
# instruction; the image's antenv lacks the NTFF profile hook shim).
# ---------------------------------------------------------------------------

def _patch_tile_drain():
    import concourse.mybir as mybir
    import concourse.tile as tile_mod
    from concourse.vector_clock import ScopedClock

    if getattr(tile_mod.TileContext, "_drain_patched", False):
        return

    def _drain_and_barrier(self, tick_clock, wait_clock):
        nc = self.nc
        probe = nc.sync.nop()
        wait_clock.add_sem_waits(probe.ins, ScopedClock({None: tick_clock.global_clock}))
        si = probe.ins.sync_info
        waits = list(si.on_wait) if si and si.on_wait else []
        if len(waits) > 1:
            si.on_wait = waits[:1]
            for w in waits[1:]:
                extra = nc.sync.nop()
                extra.ins.sync_info = mybir.SyncInfo(on_wait=[w], on_update=[])
        nc.sync.drain()
        nc.all_engine_barrier()
        assert self.sems is not None
        popped = nc._tile_sem_poison_stack.pop()
        assert popped is self._sem_poison
        nc.clear_and_free_semaphores(list(self.sems.allocated().values()))
        nc.all_engine_barrier()

    tile_mod.TileContext._drain_and_barrier = _drain_and_barrier
    tile_mod.TileContext._drain_patched = True


def _split_sync_waits(nc, max_waits=1):
    import concourse.mybir as mybir

    cnt = 0
    for f in nc.m.functions:
        for blk in f.blocks:
            insts = blk.instructions
            new = []
            for inst in insts:
                si = inst.sync_info
                waits = list(si.on_wait) if si and si.on_wait else []
                if len(waits) > max_waits:
                    keep = waits[-max_waits:]
                    excess = waits[:-max_waits]
                    for j in range(0, len(excess), max_waits):
                        n = mybir.InstNoOp(name=f"I-ws{cnt}", ins=[], outs=[])
                        cnt += 1
                        n.engine = inst.engine
                        n.sync_info = mybir.SyncInfo(
                            on_wait=excess[j:j + max_waits], on_update=[])
                        new.append(n)
                    si.on_wait = keep
                new.append(inst)
            insts[:] = new
    return cnt


def _install_ntff_shim():
    try:
        import antenv
        if "antenv.axon_hooks" in sys.modules:
            return
        mod = types.ModuleType("antenv.axon_hooks")
        mod._hook = None
        mod.set_axon_ntff_profile_hook = lambda h: setattr(mod, "_hook", h)
        mod.get_axon_ntff_profile_hook = lambda: mod._hook
        sys.modules["antenv.axon_hooks"] = mod
        antenv.axon_hooks = mod
        from trn_agent_boot.trn_boot import _ntff_profile_via_ctypes
        mod._hook = _ntff_profile_via_ctypes("/opt/axon/libaxon_pjrt.so")
    except Exception:
        pass


# ---------------------------------------------------------------------------
# Device program (identical on all 8 cores)
# ---------------------------------------------------------------------------

def _build_nc():
    import concourse.bass as bass
    import concourse.mybir as mybir
    import concourse.tile as tile

    _patch_tile_drain()

    f32 = mybir.dt.float32
    f16 = mybir.dt.float16
    
    AF = mybir.ActivationFunctionType

    nc = bass.Bass("TRN2", target_bir_lowering=False, debug=False)

    xT = nc.dram_tensor("xT", [E, N], f16, kind="ExternalInput")
    wqkvT = nc.dram_tensor("wqkvT", [E, 768], f16, kind="ExternalInput")
    woT = nc.dram_tensor("woT", [256, E], f16, kind="ExternalInput")
    bqkv = nc.dram_tensor("bqkv", [768, 1], f32, kind="ExternalInput")
    tri = nc.dram_tensor("tri", [128, 2, 128], f16, kind="ExternalInput")
    outp = nc.dram_tensor("outp", [N, E], f32, kind="ExternalOutput")

    NB = N // 512          # 4 token blocks of 512
    NT = N // 128          # 16 token tiles of 128
    NE = E // 128          # 8 contraction chunks

    with nc.allow_low_precision(reason="fp16 matmul pipeline"), \
            tile.TileContext(nc) as tc:
        with tc.tile_pool(name="const", bufs=1) as constp, \
                tc.tile_pool(name="qk", bufs=1) as qkp, \
                tc.tile_pool(name="probs", bufs=4) as pbp, \
                tc.tile_pool(name="misc", bufs=2) as miscp, \
                tc.tile_pool(name="stage", bufs=3) as stp:

            xT_sb = constp.tile([128, NE, N], f16, tag="xT")
            wq_sb = constp.tile([128, NE, 768], f16, tag="wq")
            wo_sb = constp.tile([128, 2, E], f16, tag="wo")
            bias_sb = constp.tile([128, 6, 1], f32, tag="bias")
            tri_sb = constp.tile([128, 2, 128], f16, tag="tri")
            ones_sb = constp.tile([128, 64], f16, tag="ones")
            q_sb = qkp.tile([128, 2, N], f16, tag="q")
            k_sb = qkp.tile([128, 2, N], f16, tag="k")
            vt_sb = qkp.tile([128, NT, 256], f16, tag="vt")
            at_sb = qkp.tile([128, 2, N], f16, tag="at")

            nc.vector.memset(ones_sb[:], 1.0)
            nc.sync.dma_start(tri_sb[:], tri.ap())
            for i in range(NE):
                nc.sync.dma_start(xT_sb[:, i, :], xT.ap()[i * 128:(i + 1) * 128, :])
                nc.sync.dma_start(wq_sb[:, i, :], wqkvT.ap()[i * 128:(i + 1) * 128, :])
            for i in range(2):
                nc.sync.dma_start(wo_sb[:, i, :], woT.ap()[i * 128:(i + 1) * 128, :])
            for i in range(6):
                nc.sync.dma_start(bias_sb[:, i, :], bqkv.ap()[i * 128:(i + 1) * 128, :])

            # ---- phase 1: qkv projections --------------------------------
            # q/k: nb-inner with 4 live accumulators so consecutive matmuls
            # share the stationary operand (amortizes LDWEIGHTS).
            with tc.tile_pool(name="p1", bufs=1, space="PSUM") as p1p, \
                    tc.tile_pool(name="p1v", bufs=2, space="PSUM") as p1vp:
                for ft in range(4):      # q0 q1 k0 k1 feature tiles
                    pss = [p1p.tile([128, 512], f32, tag=f"p1_{nb}",
                                    name=f"p1_{ft}_{nb}") for nb in range(NB)]
                    for e in range(NE):
                        for nb in range(NB):
                            nc.tensor.matmul(
                                pss[nb][:],
                                wq_sb[:, e, ft * 128:(ft + 1) * 128],
                                xT_sb[:, e, nb * 512:(nb + 1) * 512],
                                start=(e == 0), stop=(e == NE - 1))
                    for nb in range(NB):
                        ts5 = slice(nb * 512, nb * 512 + 512)
                        dest = (q_sb if ft < 2 else k_sb)[:, ft % 2, ts5]
                        nc.scalar.activation(dest, pss[nb][:], AF.Identity,
                                             bias=bias_sb[:, ft, :], scale=1.0)
                for tt in range(NT):     # v token tiles
                    ps = p1vp.tile([128, 256], f32, tag="p1v")
                    for e in range(NE):
                        nc.tensor.matmul(
                            ps[:],
                            xT_sb[:, e, tt * 128:(tt + 1) * 128],
                            wq_sb[:, e, 512:768],
                            start=(e == 0), stop=(e == NE - 1))
                    nc.any.tensor_copy(vt_sb[:, tt, :], ps[:])

            # ---- phase 2: causal attention -------------------------------
            # Normalization is split: PSUM-draining copies + reciprocal run
            # right after each block's k-loop (off the PE path); the PE
            # broadcast matmuls + final multiplies are deferred into the next
            # block's emission so the reciprocal latency hides under matmuls.
            with tc.tile_pool(name="sc", bufs=2, space="PSUM") as scp, \
                    tc.tile_pool(name="pv", bufs=1, space="PSUM") as pvp, \
                    tc.tile_pool(name="den", bufs=1, space="PSUM") as denp:

                def emit_norm_b(item, pool=None, tag="sc", bufs=2):
                    jj, araw, rec = item
                    bc = (pool or scp).tile([128, 2, 512], f32, tag=tag,
                                            name=f"bc_{jj}", bufs=bufs)
                    for p in (0, 1):
                        for hh in (0, 1):
                            h = 2 * p + hh
                            nc.tensor.matmul(
                                bc[64 * hh:64 * hh + 64, p, :],
                                ones_sb[32 * h:32 * h + 1, 0:64],
                                rec[32 * h:32 * h + 1, :],
                                start=True, stop=True,
                                tile_position=(32 * h, 64 * hh))
                    js = slice(512 * jj, 512 * (jj + 1))
                    for p in (0, 1):
                        nc.vector.tensor_mul(at_sb[:, p, js], araw[p][:],
                                             bc[:, p, :])
                        nc.vector.tensor_scalar_add(at_sb[:, p, js],
                                                    at_sb[:, p, js],
                                                    bias_sb[:, 4 + p, :])

                pending = None
                for j in range(NB):
                    pv_ps = [pvp.tile([128, 512], f32, tag=f"pv{p}",
                                      name=f"pv{p}_{j}") for p in (0, 1)]
                    den_ps = denp.tile([128, 512], f32, tag="den")
                    nk = 4 * (j + 1)
                    for ik in range(nk):
                        r = ik - 4 * j
                        qoff = 128 * r if r > 0 else 0
                        qs = slice(512 * j + qoff, 512 * (j + 1))
                        first, last = ik == 0, ik == nk - 1
                        pbs = []
                        for p in (0, 1):
                            sc = scp.tile([128, 2, 512], f32, tag="sc")
                            for hh in (0, 1):
                                dsl = slice(64 * hh, 64 * hh + 64)
                                nc.tensor.matmul(
                                    sc[:, hh, qoff:512],
                                    k_sb[dsl, p, ik * 128:(ik + 1) * 128],
                                    q_sb[dsl, p, qs],
                                    start=True, stop=True)
                            pb = pbp.tile([128, 2, 512], f16, tag="pb",
                                          name=f"pb_{j}_{ik}_{p}")
                            nc.scalar.activation(pb[:, :, qoff:512],
                                                 sc[:, :, qoff:512],
                                                 AF.Exp, scale=float(D) ** -0.5)
                            if r >= 0:
                                nc.vector.tensor_mul(
                                    pb[:, :, qoff:qoff + 128],
                                    pb[:, :, qoff:qoff + 128], tri_sb[:])
                            for hh in (0, 1):
                                h = 2 * p + hh
                                nc.tensor.matmul(
                                    pv_ps[p][64 * hh:64 * hh + 64, qoff:512],
                                    vt_sb[:, ik, 64 * h:64 * h + 64],
                                    pb[:, hh, qoff:512],
                                    start=first, stop=last,
                                    tile_position=(0, 64 * hh),
                                    skip_group_check=True)
                            pbs.append(pb)
                        for h in range(4):
                            nc.tensor.matmul(
                                den_ps[32 * h:32 * h + 1, qoff:512],
                                ones_sb[:, 0:1],
                                pbs[h // 2][:, h % 2, qoff:512],
                                start=first, stop=last,
                                tile_position=(0, 32 * h),
                                skip_group_check=True)
                        if pending is not None and ik == 1:
                            emit_norm_b(pending)
                            pending = None
                    araw = []
                    for p in (0, 1):
                        ar = miscp.tile([128, 512], f32, tag=f"araw{p}",
                                        name=f"araw{p}_{j}")
                        nc.any.tensor_copy(ar[:], pv_ps[p][:])
                        araw.append(ar)
                    rec = miscp.tile([128, 512], f16, tag="rec",
                                     name=f"rec_{j}")
                    nc.vector.reciprocal(rec[0:97, :], den_ps[0:97, :])
                    pending = (j, araw, rec)

            # ---- phase 3: out-projection (partial) -----------------------
            with tc.tile_pool(name="p3", bufs=3, space="PSUM") as p3p:
                for tt in range(NT):
                    if pending is not None and tt == 2:
                        emit_norm_b(pending, pool=p3p, tag="bc", bufs=1)
                        pending = None
                    for nb2 in range(2):
                        ps = p3p.tile([128, 512], f32, tag="p3")
                        for fp in range(2):
                            nc.tensor.matmul(
                                ps[:],
                                at_sb[:, fp, tt * 128:(tt + 1) * 128],
                                wo_sb[:, fp, nb2 * 512:(nb2 + 1) * 512],
                                start=(fp == 0), stop=(fp == 1))
                        st = stp.tile([128, 512], f32, tag="st")
                        nc.any.tensor_copy(st[:], ps[:])
                        nc.sync.dma_start(
                            outp.ap()[tt * 128:(tt + 1) * 128,
                                      nb2 * 512:(nb2 + 1) * 512],
                            st[:])

    _split_sync_waits(nc)
    return nc


_NC = None


def _get_nc():
    global _NC
    if _NC is None:
        _NC = _build_nc()
    return _NC


# ---------------------------------------------------------------------------
# Host entry point
# ---------------------------------------------------------------------------

def kernel(x, qkv_w, qkv_b, out_w, out_b):
    from concourse.bass_utils import run_bass_kernel_spmd

    trace_dir = os.environ.get("BASS_KERNEL_TRACE_DIR")
    if trace_dir:
        _install_ntff_shim()

    nc = _get_nc()

    x = np.asarray(x, np.float32)
    qkv_w = np.asarray(qkv_w, np.float32)
    qkv_b = np.asarray(qkv_b, np.float32)
    out_w = np.asarray(out_w, np.float32)
    out_b = np.asarray(out_b, np.float32)

    tri_np = np.broadcast_to(np.triu(np.ones((128, 128), np.float16))[:, None, :],
        (128, 2, 128)).copy()
    in_maps = []
    for c in range(NCORES):
        b, g = divmod(c, 4)
        fs = slice(256 * g, 256 * g + 256)
        wqkvT = np.ascontiguousarray(
            np.concatenate([qkv_w[0 * E:1 * E][fs],
                            qkv_w[1 * E:2 * E][fs],
                            qkv_w[2 * E:3 * E][fs]], axis=0).T)
        bq = np.concatenate([qkv_b[0 * E:1 * E][fs],
                             qkv_b[1 * E:2 * E][fs],
                             qkv_b[2 * E:3 * E][fs]])[:, None]
        in_maps.append({
            "xT": np.ascontiguousarray(x[b].T).astype(np.float16),
            "wqkvT": wqkvT.astype(np.float16),
            "woT": np.ascontiguousarray(out_w[:, fs].T).astype(np.float16),
            "bqkv": np.ascontiguousarray(bq),
            "tri": tri_np,
        })

    kwargs = {}
    if trace_dir:
        kwargs = {"trace": True, "tmpdir": trace_dir}
    res = run_bass_kernel_spmd(nc, in_maps, core_ids=list(range(NCORES)), **kwargs)
    if trace_dir and res.exec_time_ns is not None:
        print(f"HW exec time: {res.exec_time_ns} ns")

    out = np.zeros((B, N, E), np.float32)
    for c in range(NCORES):
        out[c // 4] += res.results[c]["outp"]
    out += out_b[None, None, :]
    return out
